# revision 9
# baseline (speedup 1.0000x reference)
"""Channel-attention (CAM) Trainium2 kernel.

Reference computation (per batch b of 16):
    q   = x[b].reshape(C, HW)                  # C=512, HW=4096
    sim = q @ q.T                              # [C, C], symmetric
    attn = softmax(max(sim) - sim, axis=-1)    # == exp(min_r - sim) / Z_r
    out[b] = gamma * attn @ q + x[b]

Sharding: data-parallel over batch across 8 NeuronCores (2 batches/core).
kernel() takes full inputs, shards internally, returns the full output.

Per-core kernel design (v4):
  - All matmuls in float32r (fp32 storage, ~13-bit mantissa, full PE rate
    at N>=256). bf16 is NOT accurate enough here: the softmax is
    winner-take-all (sim entries spread sigma~64), so sim errors ~0.3 flip
    argmin rows.
  - x is host-relayouted to [NB, P, CB, HW] so each column wave loads with
    ONE DMA descriptor ([128, 4, wlen] nest) straight into a 3-D f32r qr
    tile -- the Sync engine issues descriptors at ~0.6us each, so per-mi
    loads (24/batch) starved the early transposes; 6/batch fixes it.
  - qr chunks are PE-transposed (via identity matmuls) into qT tiles
    [n, c]; each sim matmul is interleaved between single transposes so
    the long (N>=256) matmuls hide the sequencer's ~120ns/instruction
    issue cost that otherwise gates bursts of short transposes.
  - sim is symmetric: compute block rows over cols >= (0,128,256,256)
    only, and fill the 5 missing lower [128,128] blocks by PE-transposing
    their mirror blocks out of PSUM (f32, exact).
  - softmax via ACT: p = exp(min_r - sim) with accum_out producing Z in
    the same pass; rows scaled by gamma/Z (DVE), PE-transposed, and the
    identity added to the diagonal so the second matmul directly
    computes gamma*attn@q + q = out (residual folded into the matmul).
  - softmax is emitted in two halves: sm_exp (fills+min+exp = all psim
    readers) before batch-1's tile allocations recycle the PSUM banks,
    and sm_pt (scale+transpose) after batch-1's first transpose waves,
    which act as PE filler for the softmax's ACT/DVE latency. batch-1's
    mm1 is deferred until after sm_exp(0) so its PSUM writes order
    correctly behind the exp reads.
  - mm2 results are staged [128, 1024] in SBUF on a 4-deep ring and
    stored with 4KB descriptor lines so the PE never waits on a store.
  - two groups of dummy identity matmuls pre-warm the PE clock gate: one
    while the first loads land, one after the framework's preamble drain
    (the drain idles the PE ~5us, resetting the p-state ramp).
"""
import sys

if "/opt/trn_rl_repo" not in sys.path:
    sys.path.insert(0, "/opt/trn_rl_repo")

import numpy as np

B, C, H, W = 16, 512, 64, 64
HW = H * W
NCORES = 8
NB = B // NCORES          # batches per core
P = 128
CB = C // P               # 4 channel blocks
KN = HW // P              # 32 contraction chunks for sim
NJ = HW // 512            # 8 output column chunks

_BUILD_CACHE = {}


def build_bass():
    import concourse.bacc as bacc
    import concourse.tile as tile
    from concourse import mybir
    from concourse.masks import make_identity

    f32 = mybir.dt.float32
    f32r = mybir.dt.float32r
    AX = mybir.AxisListType
    ALU = mybir.AluOpType
    ACTF = mybir.ActivationFunctionType

    nc = bacc.Bacc()
    x_ext = nc.declare_dram_parameter("x", [NB, P, CB, HW], f32r,
                                      isOutput=False)
    g_ext = nc.declare_dram_parameter("gamma", [1], f32, isOutput=False)
    o_ext = nc.declare_dram_parameter("out", [NB, C, HW], f32, isOutput=True)

    # alternate PSUM->SBUF copies between ACT and DVE to balance engines
    _flip = [0]

    # column waves; first ones finer to cut startup latency
    WAVES = [(0, 256), (256, 256), (512, 512),
             (1024, 1024), (2048, 1024), (3072, 1024)]
    C0S = [0, P, 2 * P, 2 * P]  # mm1 col starts per block row (N>=256)

    with tile.TileContext(nc) as tc:
        with (
            tc.tile_pool(name="const", bufs=1) as const,
            tc.tile_pool(name="qr", bufs=2) as qrp,
            tc.tile_pool(name="qt", bufs=10) as qtp,
            tc.tile_pool(name="pp", bufs=4) as pp,
            tc.tile_pool(name="osb", bufs=4) as osb,
            tc.tile_pool(name="tri", bufs=2) as trip,
            tc.tile_pool(name="vec", bufs=6) as vec,
            tc.tile_pool(name="psA", bufs=2, space="PSUM") as psA,
            tc.tile_pool(name="psim", bufs=4, space="PSUM") as psimp,
            tc.tile_pool(name="pfeat", bufs=2, space="PSUM") as pfeat,
        ):
            def copyback(dst, src):
                if _flip[0] % 2 == 0:
                    nc.scalar.copy(dst, src)
                else:
                    nc.vector.tensor_copy(dst, src)
                _flip[0] += 1

            # batch-0 first-wave load goes first so DMA starts during preamble
            qr0 = qrp.tile([P, CB, HW], f32r, tag="qr", name="qr0")
            w00, wl0 = WAVES[0]
            for mi in range(CB):
                nc.sync.dma_start(
                    out=qr0[:, mi, w00:w00 + wl0],
                    in_=x_ext[0, :, mi, w00:w00 + wl0],
                )

            ident_f = const.tile([P, P], f32)
            make_identity(nc, ident_f)
            ident_r = const.tile([P, P], f32r)
            nc.vector.tensor_copy(ident_r[:], ident_f[:])
            gamma_sb = const.tile([P, 1], f32)
            nc.sync.dma_start(out=gamma_sb[:], in_=g_ext[:].to_broadcast([P, 1]))

            # dummy matmuls while the first loads land: warms the PE clock
            # gate so real matmuls start at full rate
            warm = psA.tile([P, C], f32, tag="psA", name="warmup")
            for i in range(8):
                nc.tensor.matmul(warm[:, :P], ident_f[:], ident_f[:],
                                 start=True, stop=True)

            def mm1_one(st, kn, mi):
                c0 = C0S[mi]
                nc.tensor.matmul(
                    st["psim"][mi][:, c0:],
                    st["qt"][kn][:, mi * P:(mi + 1) * P],
                    st["qt"][kn][:, c0:],
                    start=(kn == 0),
                    stop=(kn == KN - 1),
                )

            def phase1_more(b, st, nwaves, defer_mm1=False):
                """one load per wave + transposes interleaved with lagged
                sim matmuls (long matmuls hide sequencer issue time)."""
                qr_t, pending = st["qr"], st["pending"]
                for wi, (w0, wlen) in list(enumerate(WAVES))[
                        st["nwaves"]:nwaves]:
                    if not (b == 0 and w0 == 0):
                        if wi < 3:
                            for mi in range(CB):
                                nc.sync.dma_start(
                                    out=qr_t[:, mi, w0:w0 + wlen],
                                    in_=x_ext[b, :, mi, w0:w0 + wlen],
                                )
                        else:
                            nc.sync.dma_start(
                                out=qr_t[:, :, w0:w0 + wlen],
                                in_=x_ext[b, :, :, w0:w0 + wlen],
                            )
                    for kq in range(wlen // P):
                        kn = w0 // P + kq
                        pst = psA.tile([P, C], f32r, tag="psA")
                        for ci in range(CB):
                            nc.tensor.transpose(
                                pst[:, ci * P:(ci + 1) * P],
                                qr_t[:, ci, kn * P:(kn + 1) * P],
                                ident_r[:],
                            )
                        qt = qtp.tile([P, C], f32r, tag="qt", name=f"qt{b}_{kn}")
                        st["qt"][kn] = qt
                        copyback(qt[:], pst[:])
                        pending.append(kn)
                        if not defer_mm1 and len(pending) > 2:
                            kn_mm = pending.pop(0)
                            for mi in range(CB):
                                mm1_one(st, kn_mm, mi)
                st["nwaves"] = nwaves
                if nwaves == len(WAVES):
                    for kn in pending:
                        for mi in range(CB):
                            mm1_one(st, kn, mi)
                    pending.clear()

            def phase1_start(b, nwaves, qr_pre=None, defer_mm1=False):
                st = {"pending": [], "nwaves": 0, "qt": {}}
                if qr_pre is not None:
                    st["qr"] = qr_pre
                else:
                    st["qr"] = qrp.tile([P, CB, HW], f32r, tag="qr",
                                        name=f"qr{b}")
                st["psim"] = [psimp.tile([P, C], f32, tag="psim",
                                         name=f"psim{b}_{i}") for i in range(CB)]
                phase1_more(b, st, nwaves, defer_mm1=defer_mm1)
                return st

            def sm_exp(b, st):
                """tri fills + row min + exp(min - sim) with Z accum.

                Emits every reader of st's psim PSUM banks, so the banks
                can be safely recycled by the other batch afterwards.
                """
                psim = st["psim"]
                for (i, j) in [(1, 0), (2, 0), (2, 1), (3, 0), (3, 1)]:
                    tmp = trip.tile([P, P], f32, tag="tri")
                    copyback(tmp[:], psim[j][:, i * P:(i + 1) * P])
                    nc.tensor.transpose(
                        psim[i][:, j * P:(j + 1) * P], tmp[:], ident_f[:]
                    )
                st["p"] = []
                st["z"] = []
                for mi in range(CB):
                    mrow = vec.tile([P, 1], f32, tag="mrow")
                    nc.vector.tensor_reduce(
                        mrow[:], psim[mi][:], axis=AX.X, op=ALU.min
                    )
                    zrow = vec.tile([P, 1], f32, tag="zrow")
                    p_t = pp.tile([P, C], f32r, tag="p", bufs=4)
                    nc.scalar.activation(
                        p_t[:], psim[mi][:], ACTF.Exp,
                        bias=mrow[:], scale=-1.0, accum_out=zrow[:],
                    )
                    st["p"].append(p_t)
                    st["z"].append(zrow)

            def sm_pt(b, st):
                """rows scaled by gamma/Z, PE-transposed; lhsT = T(p*g/Z)+I."""
                ps_t = []
                for mi in range(CB):
                    rz = vec.tile([P, 1], f32, tag="rz")
                    nc.vector.reciprocal(rz[:], st["z"][mi][:])
                    rzg = vec.tile([P, 1], f32, tag="rzg")
                    nc.vector.tensor_mul(rzg[:], rz[:], gamma_sb[:])
                    p_s = pp.tile([P, C], f32r, tag="psc", bufs=4)
                    nc.gpsimd.tensor_scalar_mul(p_s[:], st["p"][mi][:], rzg[:])
                    ps_t.append(p_s)
                pt_t = []
                for kd in range(CB):
                    pst = pfeat.tile([P, C], f32r, tag="pf")
                    for ci in range(CB):
                        nc.tensor.transpose(
                            pst[:, ci * P:(ci + 1) * P],
                            ps_t[ci][:, kd * P:(kd + 1) * P],
                            ident_r[:],
                        )
                    t = pp.tile([P, C], f32r, tag="pt", bufs=8)
                    copyback(t[:], pst[:])
                    nc.gpsimd.tensor_add(
                        t[:, kd * P:(kd + 1) * P],
                        t[:, kd * P:(kd + 1) * P],
                        ident_r[:],
                    )
                    pt_t.append(t)
                st["pt"] = pt_t

            def mm2(b, st, mis, grain=1024):
                """out = (gamma*diag(1/Z)*P + I) @ q, staged stores
                (4KB lines) on a 4-deep ring so the PE never waits on a
                store DMA; the final block row stores finer to cut the
                drain tail."""
                qr_t, pt_t = st["qr"], st["pt"]
                for mi in mis:
                    stg = None
                    for nj in range(NJ):
                        if stg is None:
                            stg = osb.tile([P, grain], f32, tag=f"ot{grain}",
                                           bufs=4)
                            s0 = nj * 512
                        pf = pfeat.tile([P, 512], f32, tag="pf")
                        for kd in range(CB):
                            nc.tensor.matmul(
                                pf[:],
                                pt_t[kd][:, mi * P:(mi + 1) * P],
                                qr_t[:, kd, nj * 512:(nj + 1) * 512],
                                start=(kd == 0),
                                stop=(kd == CB - 1),
                            )
                        off = nj * 512 - s0
                        copyback(stg[:, off:off + 512], pf[:])
                        if off + 512 == grain:
                            nc.sync.dma_start(
                                out=o_ext[b, mi * P:(mi + 1) * P,
                                          s0:s0 + grain],
                                in_=stg[:],
                            )
                            stg = None

            # re-warm the PE clock gate: the framework preamble ends with a
            # drain that idles the PE ~5us after the first warmup group
            warm2 = psA.tile([P, C], f32, tag="psA", name="warmup2")
            for i in range(8):
                nc.tensor.matmul(warm2[:, :P], ident_f[:], ident_f[:],
                                 start=True, stop=True)

            # phase-interleaved emission (see module docstring)
            st0 = phase1_start(0, len(WAVES), qr_pre=qr0)
            sm_exp(0, st0)
            st1 = phase1_start(1, 3, defer_mm1=True)
            sm_pt(0, st0)
            phase1_more(1, st1, len(WAVES))
            mm2(0, st0, [0, 1])
            sm_exp(1, st1)
            sm_pt(1, st1)
            mm2(0, st0, [2, 3])
            mm2(1, st1, [0, 1, 2])
            mm2(1, st1, [3], grain=512)

    nc.finalize()
    return nc


def get_bass():
    if "nc" not in _BUILD_CACHE:
        _BUILD_CACHE["nc"] = build_bass()
    return _BUILD_CACHE["nc"]


def make_in_maps(x, gamma):
    # relayout [B, C, HW] -> [B, P, CB, HW] so each column wave is a
    # single DMA descriptor per core (see module docstring)
    x = np.asarray(x, dtype=np.float32).reshape(B, CB, P, HW)
    x = np.ascontiguousarray(x.transpose(0, 2, 1, 3))
    gamma = np.asarray(gamma, dtype=np.float32).reshape(1)
    return [
        {"x": x[i * NB:(i + 1) * NB], "gamma": gamma}
        for i in range(NCORES)
    ]


def run(x, gamma, trace=False, **trace_kwargs):
    from concourse.bass_utils import run_bass_kernel_spmd

    nc = get_bass()
    res = run_bass_kernel_spmd(
        nc, make_in_maps(x, gamma), core_ids=list(range(NCORES)),
        trace=trace, **trace_kwargs,
    )
    out = np.concatenate([res.results[i]["out"] for i in range(NCORES)], axis=0)
    return out.reshape(B, C, H, W), res


def kernel(x, gamma):
    out, _ = run(x, gamma, trace=False)
    return out


# revision 10
# speedup vs baseline: 1.3266x; 1.3266x over previous
"""Channel-attention (CAM) Trainium2 kernel.

Reference computation (per batch b of 16):
    q   = x[b].reshape(C, HW)                  # C=512, HW=4096
    sim = q @ q.T                              # [C, C], symmetric
    attn = softmax(max(sim) - sim, axis=-1)    # == exp(min_r - sim) / Z_r
    out[b] = gamma * attn @ q + x[b]

Sharding: data-parallel over batch across 8 NeuronCores (2 batches/core).
kernel() takes full inputs, shards internally, returns the full output.

Per-core kernel design (v4):
  - All matmuls in float32r (fp32 storage, ~13-bit mantissa, full PE rate
    at N>=256). bf16 is NOT accurate enough here: the softmax is
    winner-take-all (sim entries spread sigma~64), so sim errors ~0.3 flip
    argmin rows.
  - x is host-relayouted to [NB, P, CB, HW] so each column wave loads with
    ONE DMA descriptor ([128, 4, wlen] nest) straight into a 3-D f32r qr
    tile -- the Sync engine issues descriptors at ~0.6us each, so per-mi
    loads (24/batch) starved the early transposes; 6/batch fixes it.
  - qr chunks are PE-transposed (via identity matmuls) into qT tiles
    [n, c]; each sim matmul is interleaved between single transposes so
    the long (N>=256) matmuls hide the sequencer's ~120ns/instruction
    issue cost that otherwise gates bursts of short transposes.
  - sim is symmetric: compute block rows over cols >= (0,128,256,256)
    only, and fill the 5 missing lower [128,128] blocks by PE-transposing
    their mirror blocks out of PSUM (f32, exact).
  - softmax via ACT: p = exp(min_r - sim) with accum_out producing Z in
    the same pass; rows scaled by gamma/Z (DVE), PE-transposed, and the
    identity added to the diagonal so the second matmul directly
    computes gamma*attn@q + q = out (residual folded into the matmul).
  - softmax is emitted in two halves: sm_exp (fills+min+exp = all psim
    readers) before batch-1's tile allocations recycle the PSUM banks,
    and sm_pt (scale+transpose) after batch-1's first transpose waves,
    which act as PE filler for the softmax's ACT/DVE latency. batch-1's
    mm1 is deferred until after sm_exp(0) so its PSUM writes order
    correctly behind the exp reads.
  - mm2 results are staged [128, 1024] in SBUF on a 4-deep ring and
    stored with 4KB descriptor lines so the PE never waits on a store.
  - two groups of dummy identity matmuls pre-warm the PE clock gate: one
    while the first loads land, one after the framework's preamble drain
    (the drain idles the PE ~5us, resetting the p-state ramp).
"""
import sys

if "/opt/trn_rl_repo" not in sys.path:
    sys.path.insert(0, "/opt/trn_rl_repo")

import numpy as np

B, C, H, W = 16, 512, 64, 64
HW = H * W
NCORES = 8
NB = B // NCORES          # batches per core
P = 128
CB = C // P               # 4 channel blocks
KN = HW // P              # 32 contraction chunks for sim
NJ = HW // 512            # 8 output column chunks

_BUILD_CACHE = {}


def build_bass():
    import concourse.bacc as bacc
    import concourse.tile as tile
    from concourse import mybir
    from concourse.masks import make_identity

    f32 = mybir.dt.float32
    f32r = mybir.dt.float32r
    AX = mybir.AxisListType
    ALU = mybir.AluOpType
    ACTF = mybir.ActivationFunctionType

    nc = bacc.Bacc()
    x_ext = nc.declare_dram_parameter("x", [NB, P, CB, HW], f32r,
                                      isOutput=False)
    g_ext = nc.declare_dram_parameter("gamma", [1], f32, isOutput=False)
    o_ext = nc.declare_dram_parameter("out", [NB, C, HW], f32, isOutput=True)

    # alternate PSUM->SBUF copies between ACT and DVE to balance engines
    _flip = [0]

    # column waves; first ones finer to cut startup latency
    WAVES = [(0, 256), (256, 256), (512, 512),
             (1024, 1024), (2048, 1024), (3072, 1024)]
    C0S = [0, P, 2 * P, 2 * P]  # mm1 col starts per block row (N>=256)

    with tile.TileContext(nc) as tc:
        with (
            tc.tile_pool(name="const", bufs=1) as const,
            tc.tile_pool(name="qr", bufs=2) as qrp,
            tc.tile_pool(name="qt", bufs=10) as qtp,
            tc.tile_pool(name="pp", bufs=4) as pp,
            tc.tile_pool(name="osb", bufs=4) as osb,
            tc.tile_pool(name="tri", bufs=2) as trip,
            tc.tile_pool(name="vec", bufs=6) as vec,
            tc.tile_pool(name="psA", bufs=2, space="PSUM") as psA,
            tc.tile_pool(name="psim", bufs=4, space="PSUM") as psimp,
            tc.tile_pool(name="pfeat", bufs=2, space="PSUM") as pfeat,
        ):
            def copyback(dst, src):
                if _flip[0] % 2 == 0:
                    nc.scalar.copy(dst, src)
                else:
                    nc.vector.tensor_copy(dst, src)
                _flip[0] += 1

            # batch-0 first-wave load goes first so DMA starts during preamble
            qr0 = qrp.tile([P, CB, HW], f32r, tag="qr", name="qr0")
            w00, wl0 = WAVES[0]
            for mi in range(CB):
                nc.sync.dma_start(
                    out=qr0[:, mi, w00:w00 + wl0],
                    in_=x_ext[0, :, mi, w00:w00 + wl0],
                )

            ident_f = const.tile([P, P], f32)
            make_identity(nc, ident_f)
            ident_r = const.tile([P, P], f32r)
            nc.vector.tensor_copy(ident_r[:], ident_f[:])
            gamma_sb = const.tile([P, 1], f32)
            nc.sync.dma_start(out=gamma_sb[:], in_=g_ext[:].to_broadcast([P, 1]))

            # dummy matmuls while the first loads land: warms the PE clock
            # gate so real matmuls start at full rate
            warm = psA.tile([P, C], f32, tag="psA", name="warmup")
            for i in range(8):
                nc.tensor.matmul(warm[:, :P], ident_f[:], ident_f[:],
                                 start=True, stop=True)

            def mm1_one(st, kn, mi):
                c0 = C0S[mi]
                nc.tensor.matmul(
                    st["psim"][mi][:, c0:],
                    st["qt"][kn][:, mi * P:(mi + 1) * P],
                    st["qt"][kn][:, c0:],
                    start=(kn == 0),
                    stop=(kn == KN - 1),
                )

            def phase1_more(b, st, nwaves, defer_mm1=False):
                """one load per wave + transposes interleaved with lagged
                sim matmuls (long matmuls hide sequencer issue time)."""
                qr_t, pending = st["qr"], st["pending"]
                for wi, (w0, wlen) in list(enumerate(WAVES))[
                        st["nwaves"]:nwaves]:
                    if not (b == 0 and w0 == 0):
                        if wi < 3:
                            for mi in range(CB):
                                nc.sync.dma_start(
                                    out=qr_t[:, mi, w0:w0 + wlen],
                                    in_=x_ext[b, :, mi, w0:w0 + wlen],
                                )
                        else:
                            nc.sync.dma_start(
                                out=qr_t[:, :, w0:w0 + wlen],
                                in_=x_ext[b, :, :, w0:w0 + wlen],
                            )
                    for kq in range(wlen // P):
                        kn = w0 // P + kq
                        pst = psA.tile([P, C], f32r, tag="psA")
                        for ci in range(CB):
                            nc.tensor.transpose(
                                pst[:, ci * P:(ci + 1) * P],
                                qr_t[:, ci, kn * P:(kn + 1) * P],
                                ident_r[:],
                            )
                        qt = qtp.tile([P, C], f32r, tag="qt", name=f"qt{b}_{kn}")
                        st["qt"][kn] = qt
                        copyback(qt[:], pst[:])
                        pending.append(kn)
                        if not defer_mm1 and len(pending) > 2:
                            kn_mm = pending.pop(0)
                            for mi in range(CB):
                                mm1_one(st, kn_mm, mi)
                st["nwaves"] = nwaves
                if nwaves == len(WAVES):
                    for kn in pending:
                        for mi in range(CB):
                            mm1_one(st, kn, mi)
                    pending.clear()

            def phase1_start(b, nwaves, qr_pre=None, defer_mm1=False):
                st = {"pending": [], "nwaves": 0, "qt": {}}
                if qr_pre is not None:
                    st["qr"] = qr_pre
                else:
                    st["qr"] = qrp.tile([P, CB, HW], f32r, tag="qr",
                                        name=f"qr{b}")
                st["psim"] = [psimp.tile([P, C], f32, tag="psim",
                                         name=f"psim{b}_{i}") for i in range(CB)]
                phase1_more(b, st, nwaves, defer_mm1=defer_mm1)
                return st

            def sm_exp(b, st):
                """tri fills + row min + exp(min - sim) with Z accum.

                Emits every reader of st's psim PSUM banks, so the banks
                can be safely recycled by the other batch afterwards.
                """
                psim = st["psim"]
                for (i, j) in [(1, 0), (2, 0), (2, 1), (3, 0), (3, 1)]:
                    tmp = trip.tile([P, P], f32, tag="tri")
                    copyback(tmp[:], psim[j][:, i * P:(i + 1) * P])
                    nc.tensor.transpose(
                        psim[i][:, j * P:(j + 1) * P], tmp[:], ident_f[:]
                    )
                st["p"] = []
                st["z"] = []
                for mi in range(CB):
                    mrow = vec.tile([P, 1], f32, tag="mrow")
                    nc.vector.tensor_reduce(
                        mrow[:], psim[mi][:], axis=AX.X, op=ALU.min
                    )
                    zrow = vec.tile([P, 1], f32, tag="zrow")
                    p_t = pp.tile([P, C], f32r, tag="p", bufs=4)
                    nc.scalar.activation(
                        p_t[:], psim[mi][:], ACTF.Exp,
                        bias=mrow[:], scale=-1.0, accum_out=zrow[:],
                    )
                    st["p"].append(p_t)
                    st["z"].append(zrow)

            def sm_pt(b, st):
                """rows scaled by gamma/Z, PE-transposed; lhsT = T(p*g/Z)+I."""
                ps_t = []
                for mi in range(CB):
                    rz = vec.tile([P, 1], f32, tag="rz")
                    nc.vector.reciprocal(rz[:], st["z"][mi][:])
                    rzg = vec.tile([P, 1], f32, tag="rzg")
                    nc.vector.tensor_mul(rzg[:], rz[:], gamma_sb[:])
                    p_s = pp.tile([P, C], f32r, tag="psc", bufs=4)
                    nc.vector.tensor_scalar_mul(p_s[:], st["p"][mi][:], rzg[:])
                    ps_t.append(p_s)
                pt_t = []
                for kd in range(CB):
                    pst = pfeat.tile([P, C], f32r, tag="pf")
                    for ci in range(CB):
                        nc.tensor.transpose(
                            pst[:, ci * P:(ci + 1) * P],
                            ps_t[ci][:, kd * P:(kd + 1) * P],
                            ident_r[:],
                        )
                    t = pp.tile([P, C], f32r, tag="pt", bufs=8)
                    copyback(t[:], pst[:])
                    nc.vector.tensor_add(
                        t[:, kd * P:(kd + 1) * P],
                        t[:, kd * P:(kd + 1) * P],
                        ident_r[:],
                    )
                    pt_t.append(t)
                st["pt"] = pt_t

            def mm2(b, st, mis, grain=1024):
                """out = (gamma*diag(1/Z)*P + I) @ q, staged stores
                (4KB lines) on a 4-deep ring so the PE never waits on a
                store DMA; the final block row stores finer to cut the
                drain tail."""
                qr_t, pt_t = st["qr"], st["pt"]
                for mi in mis:
                    stg = None
                    for nj in range(NJ):
                        if stg is None:
                            stg = osb.tile([P, grain], f32, tag=f"ot{grain}",
                                           bufs=4)
                            s0 = nj * 512
                        pf = pfeat.tile([P, 512], f32, tag="pf")
                        for kd in range(CB):
                            nc.tensor.matmul(
                                pf[:],
                                pt_t[kd][:, mi * P:(mi + 1) * P],
                                qr_t[:, kd, nj * 512:(nj + 1) * 512],
                                start=(kd == 0),
                                stop=(kd == CB - 1),
                            )
                        off = nj * 512 - s0
                        copyback(stg[:, off:off + 512], pf[:])
                        if off + 512 == grain:
                            nc.sync.dma_start(
                                out=o_ext[b, mi * P:(mi + 1) * P,
                                          s0:s0 + grain],
                                in_=stg[:],
                            )
                            stg = None

            # re-warm the PE clock gate: the framework preamble ends with a
            # drain that idles the PE ~5us after the first warmup group
            warm2 = psA.tile([P, C], f32, tag="psA", name="warmup2")
            for i in range(8):
                nc.tensor.matmul(warm2[:, :P], ident_f[:], ident_f[:],
                                 start=True, stop=True)

            # phase-interleaved emission (see module docstring)
            st0 = phase1_start(0, len(WAVES), qr_pre=qr0)
            sm_exp(0, st0)
            st1 = phase1_start(1, 3, defer_mm1=True)
            sm_pt(0, st0)
            phase1_more(1, st1, len(WAVES))
            mm2(0, st0, [0, 1])
            sm_exp(1, st1)
            sm_pt(1, st1)
            mm2(0, st0, [2, 3])
            mm2(1, st1, [0, 1, 2])
            mm2(1, st1, [3], grain=512)

    nc.finalize()
    return nc


def get_bass():
    if "nc" not in _BUILD_CACHE:
        _BUILD_CACHE["nc"] = build_bass()
    return _BUILD_CACHE["nc"]


def make_in_maps(x, gamma):
    # relayout [B, C, HW] -> [B, P, CB, HW] so each column wave is a
    # single DMA descriptor per core (see module docstring)
    x = np.asarray(x, dtype=np.float32).reshape(B, CB, P, HW)
    x = np.ascontiguousarray(x.transpose(0, 2, 1, 3))
    gamma = np.asarray(gamma, dtype=np.float32).reshape(1)
    return [
        {"x": x[i * NB:(i + 1) * NB], "gamma": gamma}
        for i in range(NCORES)
    ]


def run(x, gamma, trace=False, **trace_kwargs):
    from concourse.bass_utils import run_bass_kernel_spmd

    nc = get_bass()
    res = run_bass_kernel_spmd(
        nc, make_in_maps(x, gamma), core_ids=list(range(NCORES)),
        trace=trace, **trace_kwargs,
    )
    out = np.concatenate([res.results[i]["out"] for i in range(NCORES)], axis=0)
    return out.reshape(B, C, H, W), res


def kernel(x, gamma):
    out, _ = run(x, gamma, trace=False)
    return out


# revision 11
# speedup vs baseline: 1.3299x; 1.0025x over previous
"""Channel-attention (CAM) Trainium2 kernel.

Reference computation (per batch b of 16):
    q   = x[b].reshape(C, HW)                  # C=512, HW=4096
    sim = q @ q.T                              # [C, C], symmetric
    attn = softmax(max(sim) - sim, axis=-1)    # == exp(min_r - sim) / Z_r
    out[b] = gamma * attn @ q + x[b]

Sharding: data-parallel over batch across 8 NeuronCores (2 batches/core).
kernel() takes full inputs, shards internally, returns the full output.

Per-core kernel design:
  - All matmuls in float32r (fp32 storage, ~13-bit mantissa, full PE rate
    at N>=256). bf16 is NOT accurate enough here: the softmax is
    winner-take-all (sim entries spread sigma~64), so sim errors ~0.3 flip
    argmin rows.
  - x is host-relayouted to [NB, P, CB, HW] so the big column waves load
    with ONE DMA descriptor ([128, 4, wlen] nest) straight into a 3-D
    f32r qr tile -- the Sync engine issues descriptors at ~0.6us each, so
    per-mi loads (24/batch) starved the early transposes. The first two
    (256-col) waves still load per-mi: 4 small transfers land on 4 queues
    in parallel, minimizing time-to-first-transpose.
  - qr chunks are PE-transposed (via identity matmuls) into qT tiles
    [n, c]; sim matmul groups run two transpose-chunks behind, so
    DMA/transpose/matmul pipeline. Grouping [4xT][4xmm1] (not per-
    instruction interleave) matters: a short transpose directly before a
    matmul stalls the matmul's ~107ns stationary load, which otherwise
    hides under the preceding long matmul.
  - sim is symmetric: compute block rows over cols >= (0,128,256,256)
    only, and fill the 5 missing lower [128,128] blocks by PE-transposing
    their mirror blocks out of PSUM (f32, exact).
  - softmax via ACT: p = exp(min_r - sim) with accum_out producing Z in
    the same pass; rows scaled by gamma/Z (DVE), PE-transposed, and the
    identity added to the diagonal so the second matmul directly
    computes gamma*attn@q + q = out (residual folded into the matmul).
  - softmax is emitted in two halves: sm_exp (fills+min+exp = all psim
    readers) before batch-1's tile allocations recycle the PSUM banks,
    and sm_pt (scale+transpose) after batch-1's first transpose waves,
    which act as PE filler for the softmax's ACT/DVE latency. batch-1's
    mm1 is deferred until after sm_exp(0) so its PSUM writes order
    correctly behind the exp reads.
  - mm2 results are staged [128, 1024] in SBUF on a 4-deep ring and
    stored with 4KB descriptor lines so the PE never waits on a store.
  - two groups of dummy identity matmuls pre-warm the PE clock gate: one
    while the first loads land, one after the framework's preamble drain
    (the drain idles the PE ~5us, resetting the p-state ramp).
"""
import sys

if "/opt/trn_rl_repo" not in sys.path:
    sys.path.insert(0, "/opt/trn_rl_repo")

import numpy as np

B, C, H, W = 16, 512, 64, 64
HW = H * W
NCORES = 8
NB = B // NCORES          # batches per core
P = 128
CB = C // P               # 4 channel blocks
KN = HW // P              # 32 contraction chunks for sim
NJ = HW // 512            # 8 output column chunks

_BUILD_CACHE = {}


def build_bass():
    import concourse.bacc as bacc
    import concourse.tile as tile
    from concourse import mybir
    from concourse.masks import make_identity

    f32 = mybir.dt.float32
    f32r = mybir.dt.float32r
    AX = mybir.AxisListType
    ALU = mybir.AluOpType
    ACTF = mybir.ActivationFunctionType

    nc = bacc.Bacc()
    x_ext = nc.declare_dram_parameter("x", [NB, P, CB, HW], f32r,
                                      isOutput=False)
    g_ext = nc.declare_dram_parameter("gamma", [1], f32, isOutput=False)
    o_ext = nc.declare_dram_parameter("out", [NB, C, HW], f32, isOutput=True)

    # alternate PSUM->SBUF copies between ACT and DVE to balance engines
    _flip = [0]

    # column waves; first ones finer to cut startup latency
    WAVES = [(0, 256), (256, 256), (512, 512),
             (1024, 1024), (2048, 1024), (3072, 1024)]
    C0S = [0, P, 2 * P, 2 * P]  # mm1 col starts per block row (N>=256)

    with tile.TileContext(nc) as tc:
        with (
            tc.tile_pool(name="const", bufs=1) as const,
            tc.tile_pool(name="qr", bufs=2) as qrp,
            tc.tile_pool(name="qt", bufs=10) as qtp,
            tc.tile_pool(name="pp", bufs=4) as pp,
            tc.tile_pool(name="osb", bufs=4) as osb,
            tc.tile_pool(name="tri", bufs=2) as trip,
            tc.tile_pool(name="vec", bufs=6) as vec,
            tc.tile_pool(name="psA", bufs=2, space="PSUM") as psA,
            tc.tile_pool(name="psim", bufs=4, space="PSUM") as psimp,
            tc.tile_pool(name="pfeat", bufs=2, space="PSUM") as pfeat,
        ):
            def copyback(dst, src):
                if _flip[0] % 2 == 0:
                    nc.scalar.copy(dst, src)
                else:
                    nc.vector.tensor_copy(dst, src)
                _flip[0] += 1

            # batch-0 first-wave load goes first so DMA starts during preamble
            qr0 = qrp.tile([P, CB, HW], f32r, tag="qr", name="qr0")
            w00, wl0 = WAVES[0]
            for mi in range(CB):
                nc.sync.dma_start(
                    out=qr0[:, mi, w00:w00 + wl0],
                    in_=x_ext[0, :, mi, w00:w00 + wl0],
                )

            ident_f = const.tile([P, P], f32)
            make_identity(nc, ident_f)
            ident_r = const.tile([P, P], f32r)
            nc.vector.tensor_copy(ident_r[:], ident_f[:])
            gamma_sb = const.tile([P, 1], f32)
            nc.sync.dma_start(out=gamma_sb[:], in_=g_ext[:].to_broadcast([P, 1]))

            # dummy matmuls while the first loads land: warms the PE clock
            # gate so real matmuls start at full rate
            warm = psA.tile([P, C], f32, tag="psA", name="warmup")
            for i in range(8):
                nc.tensor.matmul(warm[:, :P], ident_f[:], ident_f[:],
                                 start=True, stop=True)

            def mm1_one(st, kn, mi):
                c0 = C0S[mi]
                nc.tensor.matmul(
                    st["psim"][mi][:, c0:],
                    st["qt"][kn][:, mi * P:(mi + 1) * P],
                    st["qt"][kn][:, c0:],
                    start=(kn == 0),
                    stop=(kn == KN - 1),
                )

            def phase1_more(b, st, nwaves, defer_mm1=False):
                """one load per wave + transposes interleaved with lagged
                sim matmuls (long matmuls hide sequencer issue time)."""
                qr_t, pending = st["qr"], st["pending"]
                for wi, (w0, wlen) in list(enumerate(WAVES))[
                        st["nwaves"]:nwaves]:
                    if not (b == 0 and w0 == 0):
                        if wi < 2:
                            for mi in range(CB):
                                nc.sync.dma_start(
                                    out=qr_t[:, mi, w0:w0 + wlen],
                                    in_=x_ext[b, :, mi, w0:w0 + wlen],
                                )
                        else:
                            nc.sync.dma_start(
                                out=qr_t[:, :, w0:w0 + wlen],
                                in_=x_ext[b, :, :, w0:w0 + wlen],
                            )
                    for kq in range(wlen // P):
                        kn = w0 // P + kq
                        pst = psA.tile([P, C], f32r, tag="psA")
                        for ci in range(CB):
                            nc.tensor.transpose(
                                pst[:, ci * P:(ci + 1) * P],
                                qr_t[:, ci, kn * P:(kn + 1) * P],
                                ident_r[:],
                            )
                        qt = qtp.tile([P, C], f32r, tag="qt", name=f"qt{b}_{kn}")
                        st["qt"][kn] = qt
                        copyback(qt[:], pst[:])
                        pending.append(kn)
                        if not defer_mm1 and len(pending) > 2:
                            kn_mm = pending.pop(0)
                            for mi in range(CB):
                                mm1_one(st, kn_mm, mi)
                st["nwaves"] = nwaves
                if nwaves == len(WAVES):
                    for kn in pending:
                        for mi in range(CB):
                            mm1_one(st, kn, mi)
                    pending.clear()

            def phase1_start(b, nwaves, qr_pre=None, defer_mm1=False):
                st = {"pending": [], "nwaves": 0, "qt": {}}
                if qr_pre is not None:
                    st["qr"] = qr_pre
                else:
                    st["qr"] = qrp.tile([P, CB, HW], f32r, tag="qr",
                                        name=f"qr{b}")
                st["psim"] = [psimp.tile([P, C], f32, tag="psim",
                                         name=f"psim{b}_{i}") for i in range(CB)]
                phase1_more(b, st, nwaves, defer_mm1=defer_mm1)
                return st

            def sm_exp(b, st):
                """tri fills + row min + exp(min - sim) with Z accum.

                Emits every reader of st's psim PSUM banks, so the banks
                can be safely recycled by the other batch afterwards.
                """
                psim = st["psim"]
                for (i, j) in [(1, 0), (2, 0), (2, 1), (3, 0), (3, 1)]:
                    tmp = trip.tile([P, P], f32, tag="tri")
                    copyback(tmp[:], psim[j][:, i * P:(i + 1) * P])
                    nc.tensor.transpose(
                        psim[i][:, j * P:(j + 1) * P], tmp[:], ident_f[:]
                    )
                st["p"] = []
                st["z"] = []
                for mi in range(CB):
                    mrow = vec.tile([P, 1], f32, tag="mrow")
                    nc.vector.tensor_reduce(
                        mrow[:], psim[mi][:], axis=AX.X, op=ALU.min
                    )
                    zrow = vec.tile([P, 1], f32, tag="zrow")
                    p_t = pp.tile([P, C], f32r, tag="p", bufs=4)
                    nc.scalar.activation(
                        p_t[:], psim[mi][:], ACTF.Exp,
                        bias=mrow[:], scale=-1.0, accum_out=zrow[:],
                    )
                    st["p"].append(p_t)
                    st["z"].append(zrow)

            def sm_pt(b, st):
                """rows scaled by gamma/Z, PE-transposed; lhsT = T(p*g/Z)+I."""
                ps_t = []
                for mi in range(CB):
                    rz = vec.tile([P, 1], f32, tag="rz")
                    nc.vector.reciprocal(rz[:], st["z"][mi][:])
                    rzg = vec.tile([P, 1], f32, tag="rzg")
                    nc.vector.tensor_mul(rzg[:], rz[:], gamma_sb[:])
                    p_s = pp.tile([P, C], f32r, tag="psc", bufs=4)
                    nc.vector.tensor_scalar_mul(p_s[:], st["p"][mi][:], rzg[:])
                    ps_t.append(p_s)
                pt_t = []
                for kd in range(CB):
                    pst = pfeat.tile([P, C], f32r, tag="pf")
                    for ci in range(CB):
                        nc.tensor.transpose(
                            pst[:, ci * P:(ci + 1) * P],
                            ps_t[ci][:, kd * P:(kd + 1) * P],
                            ident_r[:],
                        )
                    t = pp.tile([P, C], f32r, tag="pt", bufs=8)
                    copyback(t[:], pst[:])
                    nc.vector.tensor_add(
                        t[:, kd * P:(kd + 1) * P],
                        t[:, kd * P:(kd + 1) * P],
                        ident_r[:],
                    )
                    pt_t.append(t)
                st["pt"] = pt_t

            def mm2(b, st, mis, grain=1024):
                """out = (gamma*diag(1/Z)*P + I) @ q, staged stores
                (4KB lines) on a 4-deep ring so the PE never waits on a
                store DMA; the final block row stores finer to cut the
                drain tail."""
                qr_t, pt_t = st["qr"], st["pt"]
                for mi in mis:
                    stg = None
                    for nj in range(NJ):
                        if stg is None:
                            stg = osb.tile([P, grain], f32, tag=f"ot{grain}",
                                           bufs=4)
                            s0 = nj * 512
                        pf = pfeat.tile([P, 512], f32, tag="pf")
                        for kd in range(CB):
                            nc.tensor.matmul(
                                pf[:],
                                pt_t[kd][:, mi * P:(mi + 1) * P],
                                qr_t[:, kd, nj * 512:(nj + 1) * 512],
                                start=(kd == 0),
                                stop=(kd == CB - 1),
                            )
                        off = nj * 512 - s0
                        copyback(stg[:, off:off + 512], pf[:])
                        if off + 512 == grain:
                            nc.sync.dma_start(
                                out=o_ext[b, mi * P:(mi + 1) * P,
                                          s0:s0 + grain],
                                in_=stg[:],
                            )
                            stg = None

            # re-warm the PE clock gate: the framework preamble ends with a
            # drain that idles the PE ~5us after the first warmup group
            warm2 = psA.tile([P, C], f32, tag="psA", name="warmup2")
            for i in range(8):
                nc.tensor.matmul(warm2[:, :P], ident_f[:], ident_f[:],
                                 start=True, stop=True)

            # phase-interleaved emission (see module docstring)
            st0 = phase1_start(0, len(WAVES), qr_pre=qr0)
            sm_exp(0, st0)
            st1 = phase1_start(1, 3, defer_mm1=True)
            sm_pt(0, st0)
            phase1_more(1, st1, len(WAVES))
            mm2(0, st0, [0, 1])
            sm_exp(1, st1)
            sm_pt(1, st1)
            mm2(0, st0, [2, 3])
            mm2(1, st1, [0, 1, 2])
            mm2(1, st1, [3], grain=512)

    nc.finalize()
    return nc


def get_bass():
    if "nc" not in _BUILD_CACHE:
        _BUILD_CACHE["nc"] = build_bass()
    return _BUILD_CACHE["nc"]


def make_in_maps(x, gamma):
    # relayout [B, C, HW] -> [B, P, CB, HW] so each column wave is a
    # single DMA descriptor per core (see module docstring)
    x = np.asarray(x, dtype=np.float32).reshape(B, CB, P, HW)
    x = np.ascontiguousarray(x.transpose(0, 2, 1, 3))
    gamma = np.asarray(gamma, dtype=np.float32).reshape(1)
    return [
        {"x": x[i * NB:(i + 1) * NB], "gamma": gamma}
        for i in range(NCORES)
    ]


def run(x, gamma, trace=False, **trace_kwargs):
    from concourse.bass_utils import run_bass_kernel_spmd

    nc = get_bass()
    res = run_bass_kernel_spmd(
        nc, make_in_maps(x, gamma), core_ids=list(range(NCORES)),
        trace=trace, **trace_kwargs,
    )
    out = np.concatenate([res.results[i]["out"] for i in range(NCORES)], axis=0)
    return out.reshape(B, C, H, W), res


def kernel(x, gamma):
    out, _ = run(x, gamma, trace=False)
    return out


# revision 12
# speedup vs baseline: 1.3664x; 1.0275x over previous
"""Channel-attention (CAM) Trainium2 kernel.

Reference computation (per batch b of 16):
    q   = x[b].reshape(C, HW)                  # C=512, HW=4096
    sim = q @ q.T                              # [C, C], symmetric
    attn = softmax(max(sim) - sim, axis=-1)    # == exp(min_r - sim) / Z_r
    out[b] = gamma * attn @ q + x[b]

Sharding: data-parallel over batch across 8 NeuronCores (2 batches/core).
kernel() takes full inputs, shards internally, returns the full output.

Per-core kernel design:
  - All matmuls in float32r (fp32 storage, ~13-bit mantissa, full PE rate
    at N>=256). bf16 is NOT accurate enough here: the softmax is
    winner-take-all (sim entries spread sigma~64), so sim errors ~0.3 flip
    argmin rows.
  - x is host-relayouted to [NB, P, CB, HW] so the big column waves load
    with ONE DMA descriptor ([128, 4, wlen] nest) straight into a 3-D
    f32r qr tile -- the Sync engine issues descriptors at ~0.6us each, so
    per-mi loads (24/batch) starved the early transposes. The first two
    (256-col) waves still load per-mi: 4 small transfers land on 4 queues
    in parallel, minimizing time-to-first-transpose.
  - qr chunks are PE-transposed (via identity matmuls) into qT tiles
    [n, c]; sim matmul groups run two transpose-chunks behind, so
    DMA/transpose/matmul pipeline. Grouping [4xT][4xmm1] (not per-
    instruction interleave) matters: a short transpose directly before a
    matmul stalls the matmul's ~107ns stationary load, which otherwise
    hides under the preceding long matmul.
  - sim is symmetric: compute block rows over cols >= (0,128,256,256)
    only, and fill the 5 missing lower [128,128] blocks by PE-transposing
    their mirror blocks out of PSUM (f32, exact).
  - softmax via ACT: p = exp(min_r - sim) with accum_out producing Z in
    the same pass; rows scaled by gamma/Z (DVE), PE-transposed, and the
    identity added to the diagonal so the second matmul directly
    computes gamma*attn@q + q = out (residual folded into the matmul).
  - softmax is emitted in two halves: sm_exp (fills+min+exp = all psim
    readers) before batch-1's tile allocations recycle the PSUM banks,
    and sm_pt (scale+transpose) after batch-1's first transpose waves,
    which act as PE filler for the softmax's ACT/DVE latency. batch-1's
    mm1 is deferred until after sm_exp(0) so its PSUM writes order
    correctly behind the exp reads.
  - mm2 results are staged [128, 1024] in SBUF on a 4-deep ring and
    stored with 4KB descriptor lines so the PE never waits on a store.
  - two groups of dummy identity matmuls pre-warm the PE clock gate: one
    while the first loads land, one after the framework's preamble drain
    (the drain idles the PE ~5us, resetting the p-state ramp).
"""
import sys

if "/opt/trn_rl_repo" not in sys.path:
    sys.path.insert(0, "/opt/trn_rl_repo")

import numpy as np

B, C, H, W = 16, 512, 64, 64
HW = H * W
NCORES = 8
NB = B // NCORES          # batches per core
P = 128
CB = C // P               # 4 channel blocks
KN = HW // P              # 32 contraction chunks for sim
NJ = HW // 512            # 8 output column chunks

_BUILD_CACHE = {}


def build_bass():
    import concourse.bacc as bacc
    import concourse.tile as tile
    from concourse import mybir
    from concourse.masks import make_identity

    f32 = mybir.dt.float32
    f32r = mybir.dt.float32r
    AX = mybir.AxisListType
    ALU = mybir.AluOpType
    ACTF = mybir.ActivationFunctionType

    nc = bacc.Bacc()
    x_ext = nc.declare_dram_parameter("x", [NB, P, CB, HW], f32r,
                                      isOutput=False)
    g_ext = nc.declare_dram_parameter("gamma", [1], f32, isOutput=False)
    o_ext = nc.declare_dram_parameter("out", [NB, C, HW], f32, isOutput=True)

    # alternate PSUM->SBUF copies between ACT and DVE to balance engines
    _flip = [0]

    # column waves; first ones finer to cut startup latency
    WAVES = [(0, 256), (256, 256), (512, 512),
             (1024, 1024), (2048, 1024), (3072, 1024)]
    C0S = [0, P, 2 * P, 2 * P]  # mm1 col starts per block row (N>=256)

    with tile.TileContext(nc) as tc:
        with (
            tc.tile_pool(name="const", bufs=1) as const,
            tc.tile_pool(name="qr", bufs=2) as qrp,
            tc.tile_pool(name="qt", bufs=10) as qtp,
            tc.tile_pool(name="pp", bufs=4) as pp,
            tc.tile_pool(name="osb", bufs=4) as osb,
            tc.tile_pool(name="tri", bufs=2) as trip,
            tc.tile_pool(name="vec", bufs=6) as vec,
            tc.tile_pool(name="psA", bufs=2, space="PSUM") as psA,
            tc.tile_pool(name="psim", bufs=4, space="PSUM") as psimp,
            tc.tile_pool(name="pfeat", bufs=2, space="PSUM") as pfeat,
        ):
            def copyback(dst, src):
                if _flip[0] % 2 == 0:
                    nc.scalar.copy(dst, src)
                else:
                    nc.vector.tensor_copy(dst, src)
                _flip[0] += 1

            # batch-0 first-wave load goes first so DMA starts during preamble
            qr0 = qrp.tile([P, CB, HW], f32r, tag="qr", name="qr0")
            w00, wl0 = WAVES[0]
            for mi in range(CB):
                nc.sync.dma_start(
                    out=qr0[:, mi, w00:w00 + wl0],
                    in_=x_ext[0, :, mi, w00:w00 + wl0],
                )

            ident_f = const.tile([P, P], f32)
            make_identity(nc, ident_f)
            ident_r = const.tile([P, P], f32r)
            nc.vector.tensor_copy(ident_r[:], ident_f[:])
            gamma_sb = const.tile([P, 1], f32)
            nc.sync.dma_start(out=gamma_sb[:], in_=g_ext[:].to_broadcast([P, 1]))

            # dummy matmuls while the first loads land: warms the PE clock
            # gate so real matmuls start at full rate
            warm = psA.tile([P, C], f32, tag="psA", name="warmup")
            for i in range(8):
                nc.tensor.matmul(warm[:, :P], ident_f[:], ident_f[:],
                                 start=True, stop=True)

            def mm1_one(st, kn, mi):
                c0 = C0S[mi]
                nc.tensor.matmul(
                    st["psim"][mi][:, c0:],
                    st["qt"][kn][:, mi * P:(mi + 1) * P],
                    st["qt"][kn][:, c0:],
                    start=(kn == 0),
                    stop=(kn == KN - 1),
                )

            def phase1_more(b, st, nwaves, defer_mm1=False):
                """one load per wave + transposes interleaved with lagged
                sim matmuls (long matmuls hide sequencer issue time)."""
                qr_t, pending = st["qr"], st["pending"]
                for wi, (w0, wlen) in list(enumerate(WAVES))[
                        st["nwaves"]:nwaves]:
                    if not (b == 0 and w0 == 0):
                        if wi < 3:
                            for mi in range(CB):
                                nc.sync.dma_start(
                                    out=qr_t[:, mi, w0:w0 + wlen],
                                    in_=x_ext[b, :, mi, w0:w0 + wlen],
                                )
                        else:
                            for h0 in range(w0, w0 + wlen, 512):
                                nc.sync.dma_start(
                                    out=qr_t[:, :, h0:h0 + 512],
                                    in_=x_ext[b, :, :, h0:h0 + 512],
                                )
                    if b == 0 and wi == 2:
                        for _ in range(10):
                            nc.tensor.matmul(_fill["t"][:, :P], ident_f[:],
                                             ident_f[:], start=True, stop=True)
                    for kq in range(wlen // P):
                        kn = w0 // P + kq
                        pst = psA.tile([P, C], f32r, tag="psA")
                        for ci in range(CB):
                            nc.tensor.transpose(
                                pst[:, ci * P:(ci + 1) * P],
                                qr_t[:, ci, kn * P:(kn + 1) * P],
                                ident_r[:],
                            )
                        qt = qtp.tile([P, C], f32r, tag="qt", name=f"qt{b}_{kn}")
                        st["qt"][kn] = qt
                        copyback(qt[:], pst[:])
                        pending.append(kn)
                        if not defer_mm1 and len(pending) > 2:
                            kn_mm = pending.pop(0)
                            for mi in range(CB):
                                mm1_one(st, kn_mm, mi)
                st["nwaves"] = nwaves
                if nwaves == len(WAVES):
                    for kn in pending:
                        for mi in range(CB):
                            mm1_one(st, kn, mi)
                    pending.clear()

            def phase1_start(b, nwaves, qr_pre=None, defer_mm1=False):
                st = {"pending": [], "nwaves": 0, "qt": {}}
                if qr_pre is not None:
                    st["qr"] = qr_pre
                else:
                    st["qr"] = qrp.tile([P, CB, HW], f32r, tag="qr",
                                        name=f"qr{b}")
                st["psim"] = [psimp.tile([P, C], f32, tag="psim",
                                         name=f"psim{b}_{i}") for i in range(CB)]
                phase1_more(b, st, nwaves, defer_mm1=defer_mm1)
                return st

            def sm_exp(b, st):
                """tri fills + row min + exp(min - sim) with Z accum.

                Emits every reader of st's psim PSUM banks, so the banks
                can be safely recycled by the other batch afterwards.
                """
                psim = st["psim"]
                for (i, j) in [(1, 0), (2, 0), (2, 1), (3, 0), (3, 1)]:
                    tmp = trip.tile([P, P], f32, tag="tri")
                    copyback(tmp[:], psim[j][:, i * P:(i + 1) * P])
                    nc.tensor.transpose(
                        psim[i][:, j * P:(j + 1) * P], tmp[:], ident_f[:]
                    )
                st["p"] = []
                st["z"] = []
                for mi in range(CB):
                    mrow = vec.tile([P, 1], f32, tag="mrow")
                    nc.vector.tensor_reduce(
                        mrow[:], psim[mi][:], axis=AX.X, op=ALU.min
                    )
                    zrow = vec.tile([P, 1], f32, tag="zrow")
                    p_t = pp.tile([P, C], f32r, tag="p", bufs=4)
                    nc.scalar.activation(
                        p_t[:], psim[mi][:], ACTF.Exp,
                        bias=mrow[:], scale=-1.0, accum_out=zrow[:],
                    )
                    st["p"].append(p_t)
                    st["z"].append(zrow)

            def sm_pt(b, st):
                """rows scaled by gamma/Z, PE-transposed; lhsT = T(p*g/Z)+I."""
                ps_t = []
                for mi in range(CB):
                    rz = vec.tile([P, 1], f32, tag="rz")
                    nc.vector.reciprocal(rz[:], st["z"][mi][:])
                    rzg = vec.tile([P, 1], f32, tag="rzg")
                    nc.vector.tensor_mul(rzg[:], rz[:], gamma_sb[:])
                    p_s = pp.tile([P, C], f32r, tag="psc", bufs=4)
                    nc.vector.tensor_scalar_mul(p_s[:], st["p"][mi][:], rzg[:])
                    ps_t.append(p_s)
                pt_t = []
                for kd in range(CB):
                    pst = pfeat.tile([P, C], f32r, tag="pf")
                    for ci in range(CB):
                        nc.tensor.transpose(
                            pst[:, ci * P:(ci + 1) * P],
                            ps_t[ci][:, kd * P:(kd + 1) * P],
                            ident_r[:],
                        )
                    t = pp.tile([P, C], f32r, tag="pt", bufs=8)
                    copyback(t[:], pst[:])
                    nc.vector.tensor_add(
                        t[:, kd * P:(kd + 1) * P],
                        t[:, kd * P:(kd + 1) * P],
                        ident_r[:],
                    )
                    pt_t.append(t)
                st["pt"] = pt_t

            def mm2(b, st, mis, grain=1024):
                """out = (gamma*diag(1/Z)*P + I) @ q, staged stores
                (4KB lines) on a 4-deep ring so the PE never waits on a
                store DMA; the final block row stores finer to cut the
                drain tail."""
                qr_t, pt_t = st["qr"], st["pt"]
                for mi in mis:
                    stg = None
                    for nj in range(NJ):
                        if stg is None:
                            stg = osb.tile([P, grain], f32, tag=f"ot{grain}",
                                           bufs=4)
                            s0 = nj * 512
                        pf = pfeat.tile([P, 512], f32, tag="pf")
                        for kd in range(CB):
                            nc.tensor.matmul(
                                pf[:],
                                pt_t[kd][:, mi * P:(mi + 1) * P],
                                qr_t[:, kd, nj * 512:(nj + 1) * 512],
                                start=(kd == 0),
                                stop=(kd == CB - 1),
                            )
                        off = nj * 512 - s0
                        copyback(stg[:, off:off + 512], pf[:])
                        if off + 512 == grain:
                            nc.sync.dma_start(
                                out=o_ext[b, mi * P:(mi + 1) * P,
                                          s0:s0 + grain],
                                in_=stg[:],
                            )
                            stg = None

            # re-warm the PE clock gate: the framework preamble ends with a
            # drain that idles the PE ~5us after the first warmup group
            warm2 = psA.tile([P, C], f32, tag="psA", name="warmup2")
            for i in range(8):
                nc.tensor.matmul(warm2[:, :P], ident_f[:], ident_f[:],
                                 start=True, stop=True)
            fill_ps = pfeat.tile([P, C], f32, tag="pf", name="fillwarm")
            _fill = {"t": fill_ps}

            # phase-interleaved emission (see module docstring)
            st0 = phase1_start(0, len(WAVES), qr_pre=qr0)
            sm_exp(0, st0)
            st1 = phase1_start(1, 3, defer_mm1=True)
            sm_pt(0, st0)
            phase1_more(1, st1, len(WAVES))
            mm2(0, st0, [0, 1])
            sm_exp(1, st1)
            sm_pt(1, st1)
            mm2(0, st0, [2, 3])
            mm2(1, st1, [0, 1, 2])
            mm2(1, st1, [3], grain=512)

    nc.finalize()
    return nc


def get_bass():
    if "nc" not in _BUILD_CACHE:
        _BUILD_CACHE["nc"] = build_bass()
    return _BUILD_CACHE["nc"]


def make_in_maps(x, gamma):
    # relayout [B, C, HW] -> [B, P, CB, HW] so each column wave is a
    # single DMA descriptor per core (see module docstring)
    x = np.asarray(x, dtype=np.float32).reshape(B, CB, P, HW)
    x = np.ascontiguousarray(x.transpose(0, 2, 1, 3))
    gamma = np.asarray(gamma, dtype=np.float32).reshape(1)
    return [
        {"x": x[i * NB:(i + 1) * NB], "gamma": gamma}
        for i in range(NCORES)
    ]


def run(x, gamma, trace=False, **trace_kwargs):
    from concourse.bass_utils import run_bass_kernel_spmd

    nc = get_bass()
    res = run_bass_kernel_spmd(
        nc, make_in_maps(x, gamma), core_ids=list(range(NCORES)),
        trace=trace, **trace_kwargs,
    )
    out = np.concatenate([res.results[i]["out"] for i in range(NCORES)], axis=0)
    return out.reshape(B, C, H, W), res


def kernel(x, gamma):
    out, _ = run(x, gamma, trace=False)
    return out
